# revision 8
# baseline (speedup 1.0000x reference)
"""Trainium2 Bass kernel for nn_BModel (BinaryLinear: out = x @ sign(W).T / sqrt(in_dim)).

Strategy (data-parallel over 8 NeuronCores):
  - x [4096, 32768] f32 is sharded along batch: 512 rows per core.
  - W [100, 32768] f32 is host-transposed (pure layout marshalling) to
    wt = W.T [32768, 100] and replicated to every core; sign() is computed
    on-device.

Per-core kernel:
  - k is decomposed as k = rh*(128*128) + p*128 + j  (rh in [0,2), p = SBUF
    partition, j in [0,128)).  With this decomposition the transposed-x
    operand the TensorEngine needs (contraction on partitions) is produced
    purely by a strided DMA access pattern whose HBM-side runs are 512 B
    contiguous -- no on-chip transpose of x at all.
  - x tiles are loaded with a casting SWDGE DMA (f32 -> fp16) as
    xt[p, b, j]; sign(W) is exact in fp16 and PSUM accumulates in f32, so
    the only error is fp16 rounding of x (~2e-4 relative).
  - The matmul moving operand must be contiguous along b for full-rate
    SBUF streaming (a j-strided rhs measures 8x slower), so VectorE +
    ScalarE repack xt[p, b, j] -> xr[p, j, b] (strided read, contiguous
    write); both engines are otherwise idle.
  - sign(wt) is computed on ScalarE with the Sign activation from a
    bf16-cast copy of wt (bf16 cannot flip/zero the sign of any normal
    f32), pre-scaled by 2^64 so LUT behaviour near zero cannot matter;
    sign(0)=0 matches jnp.sign exactly.
  - Matmuls: psum[c, b] += sum_p w_sT[p, c] * xr[p, j-chunk, b],
    accumulating over all 256 (rh, j) contraction chunks; evacuated with a
    fused 1/sqrt(K) scale on ScalarE; output is written transposed
    [100, B] and the host transposes it back.
"""

import math

import numpy as np

N_CORES = 8
BATCH = 4096
K = 32768
C = 100
P = 128  # SBUF partitions
J = 128  # contiguous k elements per partition chunk (512 B f32 runs)
RH = K // (P * J)  # 2
B_PER_CORE = BATCH // N_CORES  # 512

_NC_CACHE = {}


REPACK = True


def _build_nc(b_per_core=B_PER_CORE, bn=128, x_bufs=2, repack=None):
    """Build + compile the per-core Bass program (identical on all cores)."""
    from contextlib import ExitStack

    import concourse.bass as bass
    import concourse.tile as tile
    from concourse import bacc, mybir

    f32 = mybir.dt.float32
    bf16 = mybir.dt.bfloat16
    f16 = mybir.dt.float16

    if repack is None:
        repack = REPACK
    bb_count = b_per_core // bn

    nc = bacc.Bacc(
        "TRN2",
        target_bir_lowering=False,
        debug=False,
        num_devices=N_CORES,
    )

    x = nc.dram_tensor("x", [b_per_core, K], f32, kind="ExternalInput").ap()
    wt = nc.dram_tensor("wt", [K, C], f32, kind="ExternalInput").ap()
    out_t = nc.dram_tensor("out_t", [C, b_per_core], f32, kind="ExternalOutput").ap()

    # k = rh*(P*J) + p*J + j
    x_view = x.rearrange("(bb b) (rh p j) -> bb rh p b j", bb=bb_count, rh=RH, p=P, j=J)
    wt_view = wt.rearrange("(rh p j) c -> p rh j c", rh=RH, p=P, j=J)

    scale = 1.0 / math.sqrt(K)

    WJC = 32  # j-extent of one w chunk tile
    n_wchunks = (RH * J) // WJC

    with tile.TileContext(nc) as tc, ExitStack() as ctx:
        wpool = ctx.enter_context(tc.tile_pool(name="w", bufs=1))
        wtmp_pool = ctx.enter_context(tc.tile_pool(name="wtmp", bufs=2))
        xpool = ctx.enter_context(tc.tile_pool(name="x", bufs=x_bufs))
        xrpool = ctx.enter_context(tc.tile_pool(name="xr", bufs=x_bufs))
        psum_pool = ctx.enter_context(tc.tile_pool(name="psum", bufs=2, space="PSUM"))
        opool = ctx.enter_context(tc.tile_pool(name="o", bufs=2))

        # --- W prep: per chunk t (rh = t*WJC//J, j0 = t*WJC%J):
        #     w_tiles[t][p, jj, c] = sign(wt[rh*P*J + p*J + j0+jj, c]) in fp16
        w_tiles = []
        for t in range(n_wchunks):
            rh, j0 = (t * WJC) // J, (t * WJC) % J
            wtmp = wtmp_pool.tile([P, WJC, C], bf16)
            # casting DMA f32 -> bf16 (bf16 never flips/zeroes a normal's sign)
            nc.gpsimd.dma_start(wtmp[:], wt_view[:, rh, j0 : j0 + WJC, :])
            wtile = wpool.tile([P, WJC, C], f16, tag=f"w{t}")
            # scale by 2^64 so the Sign LUT is only evaluated far from 0
            # (or at exactly 0); sign(0) = 0 matching jnp.sign.
            nc.scalar.activation(
                wtile[:],
                wtmp[:],
                mybir.ActivationFunctionType.Sign,
                scale=float(2.0**64),
            )
            w_tiles.append(wtile)

        # --- main loop
        for bb in range(bb_count):
            psum = psum_pool.tile([C, bn], f32)
            for rh in range(RH):
                xt = xpool.tile([P, bn, J], f16)
                # casting DMA (SWDGE): f32 HBM -> fp16 SBUF, transposed layout.
                # Split over b to stay under the 16384-descriptor DMA cap
                # (one descriptor per (p, b) contiguous 512 B run).
                bs = max(1, (P * bn) // 8192)
                for s in range(bs):
                    b0, b1 = s * bn // bs, (s + 1) * bn // bs
                    nc.gpsimd.dma_start(xt[:, b0:b1, :], x_view[bb, rh, :, b0:b1, :])
                # repack to xr[p, j, b] (contiguous b) for full-rate matmul
                # streaming.  ScalarE only: VectorE 2-port / 2-source ops
                # grab the SBUF port pair shared with GpSimd and starve
                # SWDGE descriptor generation for the casting x DMAs.
                if repack:
                    xr = xrpool.tile([P, J, bn], f16)
                    xt_T = xt[:].rearrange("p b j -> p j b")
                    nh = 2  # chunked so repack pipelines with the DMA
                    for h in range(nh):
                        j0, j1 = h * J // nh, (h + 1) * J // nh
                        nc.scalar.copy(xr[:, j0:j1, :], xt_T[:, j0:j1, :])
                    rhs = lambda j: xr[:, j, :]
                else:
                    rhs = lambda j: xt[:, :, j]
                for j in range(J):
                    t = (rh * J + j) // WJC
                    nc.tensor.matmul(
                        psum[:, :],
                        w_tiles[t][:, j % WJC, :],
                        rhs(j),
                        start=(rh == 0 and j == 0),
                        stop=(rh == RH - 1 and j == J - 1),
                    )
            ot = opool.tile([C, bn], f32)
            nc.scalar.activation(
                ot[:], psum[:, :], mybir.ActivationFunctionType.Copy, scale=scale
            )
            nc.sync.dma_start(out_t[:, bb * bn : (bb + 1) * bn], ot[:])

    nc.compile()
    return nc


def _get_nc(b_per_core=B_PER_CORE, bn=128, x_bufs=2):
    key = (b_per_core, bn, x_bufs, REPACK)
    if key not in _NC_CACHE:
        _NC_CACHE[key] = _build_nc(*key)
    return _NC_CACHE[key]


def kernel(x, W, **run_kwargs):
    from concourse import bass_utils

    x = np.ascontiguousarray(np.asarray(x, dtype=np.float32))
    W = np.asarray(W, dtype=np.float32)
    wt = np.ascontiguousarray(W.T)  # [K, C], pure layout change

    nc = _get_nc()
    in_maps = [
        {"x": x[c * B_PER_CORE : (c + 1) * B_PER_CORE], "wt": wt}
        for c in range(N_CORES)
    ]
    res = bass_utils.run_bass_kernel_spmd(
        nc, in_maps, core_ids=list(range(N_CORES)), **run_kwargs
    )
    out = np.concatenate([r["out_t"].T for r in res.results], axis=0)
    if run_kwargs:
        return out, res
    return out


# revision 10
# speedup vs baseline: 1.0237x; 1.0237x over previous
"""Trainium2 Bass kernel for nn_BModel (BinaryLinear: out = x @ sign(W).T / sqrt(in_dim)).

Strategy (data-parallel over 8 NeuronCores):
  - x [4096, 32768] f32 is sharded along batch: 512 rows per core.
  - W [100, 32768] f32 is host-transposed (pure layout marshalling) to
    wt = W.T [32768, 100] and replicated to every core; sign() is computed
    on-device.

Per-core kernel:
  - k is decomposed as k = rh*(128*128) + p*128 + j  (rh in [0,2), p = SBUF
    partition, j in [0,128)).  With this decomposition the transposed-x
    operand the TensorEngine needs (contraction on partitions) is produced
    purely by a strided DMA access pattern whose HBM-side runs are 512 B
    contiguous -- no on-chip transpose of x at all.
  - x tiles are loaded with a casting SWDGE DMA (f32 -> fp16) as
    xt[p, b, j]; sign(W) is exact in fp16 and PSUM accumulates in f32, so
    the only error is fp16 rounding of x (~2e-4 relative).
  - The matmul moving operand must be contiguous along b for full-rate
    SBUF streaming (a j-strided rhs measures 8x slower), so VectorE +
    ScalarE repack xt[p, b, j] -> xr[p, j, b] (strided read, contiguous
    write); both engines are otherwise idle.
  - sign(wt) is computed on ScalarE with the Sign activation from a
    bf16-cast copy of wt (bf16 cannot flip/zero the sign of any normal
    f32), pre-scaled by 2^64 so LUT behaviour near zero cannot matter;
    sign(0)=0 matches jnp.sign exactly.
  - Matmuls: psum[c, b] += sum_p w_sT[p, c] * xr[p, j-chunk, b],
    accumulating over all 256 (rh, j) contraction chunks; evacuated with a
    fused 1/sqrt(K) scale on ScalarE; output is written transposed
    [100, B] and the host transposes it back.
"""

import math

import numpy as np

N_CORES = 8
BATCH = 4096
K = 32768
C = 100
P = 128  # SBUF partitions
J = 128  # contiguous k elements per partition chunk (512 B f32 runs)
RH = K // (P * J)  # 2
B_PER_CORE = BATCH // N_CORES  # 512

_NC_CACHE = {}


REPACK = True


def _build_nc(b_per_core=B_PER_CORE, bn=128, x_bufs=2, repack=None):
    """Build + compile the per-core Bass program (identical on all cores)."""
    from contextlib import ExitStack

    import concourse.bass as bass
    import concourse.tile as tile
    from concourse import bacc, mybir

    f32 = mybir.dt.float32
    bf16 = mybir.dt.bfloat16
    f16 = mybir.dt.float16

    if repack is None:
        repack = REPACK
    bb_count = b_per_core // bn

    nc = bacc.Bacc(
        "TRN2",
        target_bir_lowering=False,
        debug=False,
        num_devices=N_CORES,
    )

    x = nc.dram_tensor("x", [b_per_core, K], f32, kind="ExternalInput").ap()
    wt = nc.dram_tensor("wt", [K, C], f32, kind="ExternalInput").ap()
    out_t = nc.dram_tensor("out_t", [C, b_per_core], f32, kind="ExternalOutput").ap()

    # k = rh*(P*J) + p*J + j
    x_view = x.rearrange("(bb b) (rh p j) -> bb rh p b j", bb=bb_count, rh=RH, p=P, j=J)
    wt_view = wt.rearrange("(rh p j) c -> p rh j c", rh=RH, p=P, j=J)

    scale = 1.0 / math.sqrt(K)

    WJC = 16  # j-extent of one w chunk tile
    n_wchunks = (RH * J) // WJC

    with tile.TileContext(nc) as tc, ExitStack() as ctx:
        wpool = ctx.enter_context(tc.tile_pool(name="w", bufs=1))
        wtmp_pool = ctx.enter_context(tc.tile_pool(name="wtmp", bufs=2))
        xpool = ctx.enter_context(tc.tile_pool(name="x", bufs=x_bufs))
        xrpool = ctx.enter_context(tc.tile_pool(name="xr", bufs=x_bufs))
        psum_pool = ctx.enter_context(tc.tile_pool(name="psum", bufs=2, space="PSUM"))
        opool = ctx.enter_context(tc.tile_pool(name="o", bufs=2))

        # --- W prep: per chunk t (rh = t*WJC//J, j0 = t*WJC%J):
        #     w_tiles[t][p, jj, c] = sign(wt[rh*P*J + p*J + j0+jj, c]) in fp16
        w_tiles = []
        for t in range(n_wchunks):
            rh, j0 = (t * WJC) // J, (t * WJC) % J
            wtmp = wtmp_pool.tile([P, WJC, C], bf16)
            # casting DMA f32 -> bf16 (bf16 never flips/zeroes a normal's sign)
            nc.gpsimd.dma_start(wtmp[:], wt_view[:, rh, j0 : j0 + WJC, :])
            wtile = wpool.tile([P, WJC, C], f16, tag=f"w{t}")
            # scale by 2^64 so the Sign LUT is only evaluated far from 0
            # (or at exactly 0); sign(0) = 0 matching jnp.sign.
            nc.scalar.activation(
                wtile[:],
                wtmp[:],
                mybir.ActivationFunctionType.Sign,
                scale=float(2.0**64),
            )
            w_tiles.append(wtile)

        # --- main loop
        for bb in range(bb_count):
            psum = psum_pool.tile([C, bn], f32)
            for rh in range(RH):
                xt = xpool.tile([P, bn, J], f16)
                # casting DMA (SWDGE): f32 HBM -> fp16 SBUF, transposed layout.
                # Split over b to stay under the 16384-descriptor DMA cap
                # (one descriptor per (p, b) contiguous 512 B run).
                bs = max(1, (P * bn) // 8192)
                for s in range(bs):
                    b0, b1 = s * bn // bs, (s + 1) * bn // bs
                    nc.gpsimd.dma_start(xt[:, b0:b1, :], x_view[bb, rh, :, b0:b1, :])
                # repack to xr[p, j, b] (contiguous b) for full-rate matmul
                # streaming.  ScalarE only: VectorE 2-port / 2-source ops
                # grab the SBUF port pair shared with GpSimd and starve
                # SWDGE descriptor generation for the casting x DMAs.
                if repack:
                    xr = xrpool.tile([P, J, bn], f16)
                    xt_T = xt[:].rearrange("p b j -> p j b")
                    # Split the repack between ScalarE and VectorE (both
                    # otherwise idle).  VectorE uses odd-inner-width copies:
                    # 127 forces 1x mode = single dedicated read port, so it
                    # cannot grab the SBUF port pair shared with GpSimd and
                    # starve SWDGE descriptor generation for the x DMAs.
                    jsplit = (J * 9) // 16  # ScalarE share (it's 1.25x faster)
                    for h in range(2):
                        j0, j1 = h * jsplit // 2, (h + 1) * jsplit // 2
                        nc.scalar.copy(xr[:, j0:j1, :], xt_T[:, j0:j1, :])
                    for h in range(2):
                        j0 = jsplit + h * (J - jsplit) // 2
                        j1 = jsplit + (h + 1) * (J - jsplit) // 2
                        nc.vector.tensor_copy(
                            xr[:, j0:j1, 0 : bn - 1], xt_T[:, j0:j1, 0 : bn - 1]
                        )
                        nc.vector.tensor_copy(
                            xr[:, j0:j1, bn - 1 : bn], xt_T[:, j0:j1, bn - 1 : bn]
                        )
                    rhs = lambda j: xr[:, j, :]
                else:
                    rhs = lambda j: xt[:, :, j]
                for j in range(J):
                    t = (rh * J + j) // WJC
                    nc.tensor.matmul(
                        psum[:, :],
                        w_tiles[t][:, j % WJC, :],
                        rhs(j),
                        start=(rh == 0 and j == 0),
                        stop=(rh == RH - 1 and j == J - 1),
                    )
            ot = opool.tile([C, bn], f32)
            nc.scalar.activation(
                ot[:], psum[:, :], mybir.ActivationFunctionType.Copy, scale=scale
            )
            nc.sync.dma_start(out_t[:, bb * bn : (bb + 1) * bn], ot[:])

    nc.compile()
    return nc


def _get_nc(b_per_core=B_PER_CORE, bn=128, x_bufs=2):
    key = (b_per_core, bn, x_bufs, REPACK)
    if key not in _NC_CACHE:
        _NC_CACHE[key] = _build_nc(*key)
    return _NC_CACHE[key]


def kernel(x, W, **run_kwargs):
    from concourse import bass_utils

    x = np.ascontiguousarray(np.asarray(x, dtype=np.float32))
    W = np.asarray(W, dtype=np.float32)
    wt = np.ascontiguousarray(W.T)  # [K, C], pure layout change

    nc = _get_nc()
    in_maps = [
        {"x": x[c * B_PER_CORE : (c + 1) * B_PER_CORE], "wt": wt}
        for c in range(N_CORES)
    ]
    res = bass_utils.run_bass_kernel_spmd(
        nc, in_maps, core_ids=list(range(N_CORES)), **run_kwargs
    )
    out = np.concatenate([r["out_t"].T for r in res.results], axis=0)
    if run_kwargs:
        return out, res
    return out


# revision 11
# speedup vs baseline: 1.0542x; 1.0298x over previous
"""Trainium2 Bass kernel for nn_BModel (BinaryLinear: out = x @ sign(W).T / sqrt(in_dim)).

Strategy (data-parallel over 8 NeuronCores):
  - x [4096, 32768] f32 is sharded along batch: 512 rows per core.
  - W [100, 32768] f32 is host-transposed (pure layout marshalling) to
    wt = W.T [32768, 100] and replicated to every core; sign() is computed
    on-device.

Per-core kernel:
  - k is decomposed as k = rh*(128*128) + p*128 + j  (rh in [0,2), p = SBUF
    partition, j in [0,128)).  With this decomposition the transposed-x
    operand the TensorEngine needs (contraction on partitions) is produced
    purely by a strided DMA access pattern whose HBM-side runs are 512 B
    contiguous -- no on-chip transpose of x at all.
  - x tiles are loaded with a casting SWDGE DMA (f32 -> fp16) as
    xt[p, b, j]; sign(W) is exact in fp16 and PSUM accumulates in f32, so
    the only error is fp16 rounding of x (~2e-4 relative).
  - The matmul moving operand must be contiguous along b for full-rate
    SBUF streaming (a j-strided rhs measures 8x slower), so VectorE +
    ScalarE repack xt[p, b, j] -> xr[p, j, b] (strided read, contiguous
    write); both engines are otherwise idle.
  - sign(wt) is computed on ScalarE with the Sign activation from a
    bf16-cast copy of wt (bf16 cannot flip/zero the sign of any normal
    f32), pre-scaled by 2^64 so LUT behaviour near zero cannot matter;
    sign(0)=0 matches jnp.sign exactly.
  - Matmuls: psum[c, b] += sum_p w_sT[p, c] * xr[p, j-chunk, b],
    accumulating over all 256 (rh, j) contraction chunks; evacuated with a
    fused 1/sqrt(K) scale on ScalarE; output is written transposed
    [100, B] and the host transposes it back.
"""

import math

import numpy as np

N_CORES = 8
BATCH = 4096
K = 32768
C = 100
P = 128  # SBUF partitions
J = 128  # contiguous k elements per partition chunk (512 B f32 runs)
RH = K // (P * J)  # 2
B_PER_CORE = BATCH // N_CORES  # 512

_NC_CACHE = {}


REPACK = True


def _build_nc(b_per_core=B_PER_CORE, bn=128, x_bufs=2, repack=None):
    """Build + compile the per-core Bass program (identical on all cores)."""
    from contextlib import ExitStack

    import concourse.bass as bass
    import concourse.tile as tile
    from concourse import bacc, mybir

    f32 = mybir.dt.float32
    bf16 = mybir.dt.bfloat16
    f16 = mybir.dt.float16

    if repack is None:
        repack = REPACK
    bb_count = b_per_core // bn

    nc = bacc.Bacc(
        "TRN2",
        target_bir_lowering=False,
        debug=False,
        num_devices=N_CORES,
    )

    x = nc.dram_tensor("x", [b_per_core, K], f32, kind="ExternalInput").ap()
    wt = nc.dram_tensor("wt", [K, C], f32, kind="ExternalInput").ap()
    out_t = nc.dram_tensor("out_t", [C, b_per_core], f32, kind="ExternalOutput").ap()

    # k = rh*(P*J) + p*J + j
    x_view = x.rearrange("(bb b) (rh p j) -> bb rh p b j", bb=bb_count, rh=RH, p=P, j=J)
    wt_view = wt.rearrange("(rh p j) c -> p rh j c", rh=RH, p=P, j=J)

    scale = 1.0 / math.sqrt(K)

    WJC = 16  # j-extent of one w chunk tile
    n_wchunks = (RH * J) // WJC

    with tile.TileContext(nc) as tc, ExitStack() as ctx:
        wpool = ctx.enter_context(tc.tile_pool(name="w", bufs=1))
        wtmp_pool = ctx.enter_context(tc.tile_pool(name="wtmp", bufs=2))
        xpool = ctx.enter_context(tc.tile_pool(name="x", bufs=x_bufs))
        xrpool = ctx.enter_context(tc.tile_pool(name="xr", bufs=x_bufs))
        psum_pool = ctx.enter_context(tc.tile_pool(name="psum", bufs=2, space="PSUM"))
        opool = ctx.enter_context(tc.tile_pool(name="o", bufs=2))

        # --- W prep, emitted lazily so the first x tiles interleave with
        #     W-chunk loads.  Per chunk t (rh = t*WJC//J, j0 = t*WJC%J):
        #     w_tiles[t][p, jj, c] = sign(wt[rh*P*J + p*J + j0+jj, c]) in fp16
        w_tiles = [None] * n_wchunks

        def emit_wchunk(t):
            rh, j0 = (t * WJC) // J, (t * WJC) % J
            wtmp = wtmp_pool.tile([P, WJC, C], bf16)
            # casting DMA f32 -> bf16 (bf16 never flips/zeroes a normal's sign)
            nc.gpsimd.dma_start(wtmp[:], wt_view[:, rh, j0 : j0 + WJC, :])
            wtile = wpool.tile([P, WJC, C], f16, tag=f"w{t}")
            # scale by 2^64 so the Sign LUT is only evaluated far from 0
            # (or at exactly 0); sign(0) = 0 matching jnp.sign.
            nc.scalar.activation(
                wtile[:],
                wtmp[:],
                mybir.ActivationFunctionType.Sign,
                scale=float(2.0**64),
            )
            w_tiles[t] = wtile

        # chunks needed by the first tile's matmuls come right after its DMA;
        # the rest follow after the second tile's DMA.
        pending_evac = []

        def emit_evac():
            psum_e, bb_e = pending_evac.pop(0)
            ot = opool.tile([C, bn], f32)
            nc.scalar.activation(
                ot[:], psum_e[:, :], mybir.ActivationFunctionType.Copy, scale=scale
            )
            nc.sync.dma_start(out_t[:, bb_e * bn : (bb_e + 1) * bn], ot[:])

        # --- main loop
        for bb in range(bb_count):
            psum = psum_pool.tile([C, bn], f32)
            for rh in range(RH):
                xt = xpool.tile([P, bn, J], f16)
                # casting DMA (SWDGE): f32 HBM -> fp16 SBUF, transposed layout.
                # Split over b to stay under the 16384-descriptor DMA cap
                # (one descriptor per (p, b) contiguous 512 B run).
                bs = max(1, (P * bn) // 8192)
                for s in range(bs):
                    b0, b1 = s * bn // bs, (s + 1) * bn // bs
                    nc.gpsimd.dma_start(xt[:, b0:b1, :], x_view[bb, rh, :, b0:b1, :])
                if bb == 0:
                    # interleave W-chunk loads behind the first x tiles
                    for t in range(rh * n_wchunks // RH, (rh + 1) * n_wchunks // RH):
                        emit_wchunk(t)
                # repack to xr[p, j, b] (contiguous b) for full-rate matmul
                # streaming.  ScalarE only: VectorE 2-port / 2-source ops
                # grab the SBUF port pair shared with GpSimd and starve
                # SWDGE descriptor generation for the casting x DMAs.
                if repack:
                    xr = xrpool.tile([P, J, bn], f16)
                    xt_T = xt[:].rearrange("p b j -> p j b")
                    # Split the repack between ScalarE and VectorE (both
                    # otherwise idle).  VectorE uses odd-inner-width copies:
                    # 127 forces 1x mode = single dedicated read port, so it
                    # cannot grab the SBUF port pair shared with GpSimd and
                    # starve SWDGE descriptor generation for the x DMAs.
                    jsplit = 44  # ScalarE share (measured ~0.57 Gelem/s
                    # vs VectorE ~1.11 Gelem/s on strided fp16 reads)
                    nc.scalar.copy(xr[:, :jsplit, :], xt_T[:, :jsplit, :])
                    nc.vector.tensor_copy(
                        xr[:, jsplit:, 0 : bn - 1], xt_T[:, jsplit:, 0 : bn - 1]
                    )
                    nc.vector.tensor_copy(
                        xr[:, jsplit:, bn - 1 : bn], xt_T[:, jsplit:, bn - 1 : bn]
                    )
                    rhs = lambda j: xr[:, j, :]
                else:
                    rhs = lambda j: xt[:, :, j]
                for j in range(J):
                    t = (rh * J + j) // WJC
                    nc.tensor.matmul(
                        psum[:, :],
                        w_tiles[t][:, j % WJC, :],
                        rhs(j),
                        start=(rh == 0 and j == 0),
                        stop=(rh == RH - 1 and j == J - 1),
                    )
            # evacuate with one-bb lag so the (in-order) ScalarE queue never
            # head-of-line-blocks the next tile's repack behind this bb's
            # matmuls.
            pending_evac.append((psum, bb))
            if len(pending_evac) > 1:
                emit_evac()
        while pending_evac:
            emit_evac()

    nc.compile()
    return nc


def _get_nc(b_per_core=B_PER_CORE, bn=128, x_bufs=2):
    key = (b_per_core, bn, x_bufs, REPACK)
    if key not in _NC_CACHE:
        _NC_CACHE[key] = _build_nc(*key)
    return _NC_CACHE[key]


def kernel(x, W, **run_kwargs):
    from concourse import bass_utils

    x = np.ascontiguousarray(np.asarray(x, dtype=np.float32))
    W = np.asarray(W, dtype=np.float32)
    wt = np.ascontiguousarray(W.T)  # [K, C], pure layout change

    nc = _get_nc()
    in_maps = [
        {"x": x[c * B_PER_CORE : (c + 1) * B_PER_CORE], "wt": wt}
        for c in range(N_CORES)
    ]
    res = bass_utils.run_bass_kernel_spmd(
        nc, in_maps, core_ids=list(range(N_CORES)), **run_kwargs
    )
    out = np.concatenate([r["out_t"].T for r in res.results], axis=0)
    if run_kwargs:
        return out, res
    return out
